# revision 4
# baseline (speedup 1.0000x reference)
"""DFMSDA block kernel for 8 Trainium2 NeuronCores.

Sharding: 8 cores = 4 batches x 2 independent attention streams (s1/s2).
Each core computes one full (batch, stream) output [C, H, W] — the two
streams share inputs but use different weights, and every op (LN, 1x1-conv
GEMMs, per-pixel 1x9 dilated window attention) is batch-independent, so no
cross-core communication is needed.
"""
import numpy as np
import jax
import jax.numpy as jnp
from functools import partial

K = 3
DILS = (1, 2, 3)
HEAD_DIM = 64
SCALE = HEAD_DIM ** -0.5
EPS = 1e-6
B, C, H, W = 4, 384, 64, 64


def _shifts(x, d):
    # x: [h, hd, H, W] -> list of 9 zero-padded shifted copies (static slices)
    h, hd, Hh, Ww = x.shape
    xp = jnp.pad(x, ((0, 0), (0, 0), (d, d), (d, d)))
    return [xp[:, :, i * d:i * d + Hh, j * d:j * d + Ww]
            for i in range(K) for j in range(K)]


def _dilate_attn(q, k, v, dil):
    # q,k,v: [dch, H, W] -> per-pixel attention over the 3x3 dilated window.
    # The 9 window offsets stay unrolled as shifted elementwise terms so no
    # gather/indirect-load is ever generated.
    dch, Hh, Ww = q.shape
    h = dch // HEAD_DIM
    qh = q.reshape(h, HEAD_DIM, Hh, Ww)
    ks = _shifts(k.reshape(h, HEAD_DIM, Hh, Ww), dil)
    vs = _shifts(v.reshape(h, HEAD_DIM, Hh, Ww), dil)
    logits = [(qh * kj).sum(axis=1) * SCALE for kj in ks]     # 9 x [h,H,W]
    m = logits[0]
    for lj in logits[1:]:
        m = jnp.maximum(m, lj)
    es = [jnp.exp(lj - m) for lj in logits]
    den = es[0]
    for ej in es[1:]:
        den = den + ej
    inv = 1.0 / den
    x = None
    for ej, vj in zip(es, vs):
        term = (ej * inv)[:, None] * vj                       # [h,hd,H,W]
        x = term if x is None else x + term
    # reference flattens as (pixel, head, hd) then views as (dch, H, W)
    return x.transpose(2, 3, 0, 1).reshape(dch, Hh, Ww)


def _mda(qm, km, vm, Wq, bq, Wk, bk, Wv, bv, Wp, bp):
    # qm,km,vm: [H,W,C]
    Hh, Ww, Cc = qm.shape
    nd = len(DILS)
    c = Cc // nd

    def proj(x, Wt, bt):
        y = jnp.einsum('hwc,oc->ohw', x, Wt) + bt[:, None, None]
        return y.reshape(nd, c, Hh, Ww)

    q = proj(qm, Wq, bq)
    k = proj(km, Wk, bk)
    v = proj(vm, Wv, bv)
    outs = [_dilate_attn(q[i], k[i], v[i], DILS[i]) for i in range(nd)]
    # stack [nd, dch, H, W] -> permute to [dch, H, nd, W] -> reshape [H,W,C]
    x = jnp.stack(outs, 0).transpose(1, 2, 0, 3).reshape(Hh, Ww, Cc)
    return x @ Wp.T + bp


def _ln(x, g, b):
    m = x.mean(-1, keepdims=True)
    v = ((x - m) ** 2).mean(-1, keepdims=True)
    return (x - m) / jnp.sqrt(v + EPS) * g + b


def _shard_fn(A, Bt, gA, bA, gB, bB, Wq, bq, Wk, bk, Wv, bv, Wp, bp):
    # A, Bt: [C, H, W].  Computes s = LN(A)-LN(Bt); out = s + MDA(s, LN(A), LN(A))
    A_f = _ln(A.transpose(1, 2, 0), gA, bA)   # [H,W,C]
    B_f = _ln(Bt.transpose(1, 2, 0), gB, bB)
    s = A_f - B_f
    out = s + _mda(s, A_f, A_f, Wq, bq, Wk, bk, Wv, bv, Wp, bp)
    return out.transpose(2, 0, 1)             # [C,H,W]


_pfn = jax.pmap(_shard_fn)


def kernel(vi, ir, g_vi, b_vi, g_ir, b_ir,
           wq1, bq1, wk1, bk1, wv1, bv1, wp1, bp1,
           wq2, bq2, wk2, bk2, wv2, bv2, wp2, bp2):
    f32 = np.float32
    vi = np.asarray(vi, f32)
    ir = np.asarray(ir, f32)

    # Device i handles (b = i // 2, stream = i % 2).
    A = np.empty((8, C, H, W), f32)
    Bt = np.empty((8, C, H, W), f32)
    for b in range(B):
        A[2 * b + 0], Bt[2 * b + 0] = vi[b], ir[b]   # stream 1: k/v from vi
        A[2 * b + 1], Bt[2 * b + 1] = ir[b], vi[b]   # stream 2: k/v from ir
    tile8 = lambda x0, x1: np.stack([np.asarray(x0 if i % 2 == 0 else x1, f32)
                                     for i in range(8)])
    args = (
        tile8(g_vi, g_ir), tile8(b_vi, b_ir),
        tile8(g_ir, g_vi), tile8(b_ir, b_vi),
        tile8(wq1, wq2), tile8(bq1, bq2),
        tile8(wk1, wk2), tile8(bk1, bk2),
        tile8(wv1, wv2), tile8(bv1, bv2),
        tile8(wp1, wp2), tile8(bp1, bp2),
    )
    out = np.asarray(_pfn(A, Bt, *args))     # [8, C, H, W]
    s1 = out[0::2]                           # [4, C, H, W]
    s2 = out[1::2]
    return (np.asarray(s1, f32), np.asarray(s2, f32))
